# revision 14
# baseline (speedup 1.0000x reference)
"""Masked cross-attention kernel for Trainium2 (8 NeuronCores, SPMD).

Problem: B=16 batches of softmax(mask(Q@K^T/sqrt(D)))@V with
Lq=Lk=2048, D=DV=256.  The reference zeroes masked scores (NOT -inf)
before the softmax, so masked keys still contribute exp(0)=1 to the
denominator and weight 1/denom on V rows.

Strategy (all host prep is exact):
  * Zero K rows at k >= valid_length[b] on the host.  Then Q @ K^T is
    *exactly* 0.0 at masked positions — identical to the reference's
    jnp.where — and no mask tensor is needed on-device.
  * Pre-transpose Q and K to [D, L] layout on the host so both matmul
    operands stream naturally (contraction on the partition dim).
  * Append a ones-column to V.  P @ [V | 1] then yields the softmax
    denominator as output column 256 for free.
  * bf16 matmul inputs (fp32 PSUM accumulate), fp32 softmax math.

Per core: 2 batches.  Per batch, for each 512-wide q tile:
  stage 1: S^T[k,q] tiles in PSUM (KtT.T @ Qt), exp via ScalarE
           (scale=1/16 folded in) -> P^T bf16 in SBUF
  stage 2: O[q,v] = (P^T).T @ [V|1] accumulated over k chunks in PSUM;
           divide by column 256 (DVE reciprocal + per-partition mul).
Stage 1 of q-tile i+1 is emitted before stage 2 of q-tile i so the PE
never stalls on the ScalarE exp chain.
"""

import numpy as np
import ml_dtypes

import concourse.bass as bass
import concourse.mybir as mybir
import concourse.tile as tile
from concourse import bacc
from concourse.bass_utils import run_bass_kernel_spmd

B, LQ, LK, D, DV = 16, 2048, 2048, 256, 256
N_CORES = 8
BPC = B // N_CORES  # batches per core

QT = 512            # q-tile width (stage-1 moving free dim)
NQT = LQ // QT      # 4
KT = 128            # k-tile (partition dim of S^T)
NKT = LK // KT      # 16
KG = 2              # k-tiles per PSUM/exp group
NKG = NKT // KG     # 8
NDC = D // 128      # contraction chunks (2)
QS = 128            # q-subtile for stage 2
NQS = QT // QS      # 4
VF = DV + 1         # 257: V plus the ones column
WARMUP_MMS = 8      # HAM warm-up zero-matmuls before the first real MM

_BF16 = mybir.dt.bfloat16
_F32 = mybir.dt.float32

_NC_CACHE = {}


def _build_nc():
    nc = bacc.Bacc("TRN2", target_bir_lowering=False, debug=False,
                   num_devices=N_CORES)

    qt_d = nc.declare_dram_parameter("qt", [BPC, D, LQ], _BF16, isOutput=False)
    kt_d = nc.declare_dram_parameter("kt", [BPC, D, LK], _BF16, isOutput=False)
    v1_d = nc.declare_dram_parameter("v1", [BPC, LK, VF], _BF16, isOutput=False)
    out_d = nc.declare_dram_parameter("out", [BPC, LQ, DV], _F32, isOutput=True)

    with tile.TileContext(nc) as tc:
        with (
            tc.tile_pool(name="qk", bufs=2) as qk_pool,
            tc.tile_pool(name="v", bufs=2) as v_pool,
            tc.tile_pool(name="p", bufs=2) as p_pool,
            tc.tile_pool(name="osb", bufs=4) as o_pool,
            tc.tile_pool(name="small", bufs=8) as small_pool,
            tc.tile_pool(name="ps_s", bufs=2, space="PSUM") as ps_s,
            tc.tile_pool(name="ps_o", bufs=4, space="PSUM") as ps_o,
        ):
            def load_batch(b):
                # Split loads so the first q-tile's operands land ASAP:
                # kt per d-chunk, qt per q-tile; v1 (stage-2 only) after
                # the first q chunk.
                # kt split [c-chunk][k-half], v1 split in k-halves, qt per
                # q-tile: chunks land in the order the PE consumes them.
                QLK = LK // 4
                kt_view = kt_d[b].rearrange("(c p) k -> p c k", p=128)
                kt_sb = [qk_pool.tile([128, NDC, QLK], _BF16, tag=f"ktq{h}",
                                      name=f"ktq{h}_b{b}") for h in range(4)]
                qt_view = qt_d[b].rearrange("(c p) q -> p c q", p=128)
                qt_sb = [qk_pool.tile([128, NDC, QT], _BF16, tag=f"qt{qi}",
                                      name=f"qt{qi}_b{b}")
                         for qi in range(NQT)]
                v1_view = v1_d[b].rearrange("(t p) v -> p t v", p=128)
                v1_sb = [v_pool.tile([128, NKT // 2, VF], _BF16, tag=f"v1{h}",
                                     name=f"v1{h}_b{b}") for h in range(2)]

                # Two parallel DMA streams: kt quarters on gpsimd (SWDGE),
                # everything else on sync (HWDGE), both deadline-ordered.
                for h in range(4):
                    nc.gpsimd.dma_start(
                        out=kt_sb[h], in_=kt_view[:, :, h * QLK:(h + 1) * QLK])
                nc.sync.dma_start(out=qt_sb[0], in_=qt_view[:, :, 0:QT])
                nc.sync.dma_start(out=qt_sb[1], in_=qt_view[:, :, QT:2 * QT])
                nc.sync.dma_start(out=v1_sb[0], in_=v1_view[:, 0:NKT // 2, :])
                nc.sync.dma_start(out=qt_sb[2], in_=qt_view[:, :, 2 * QT:3 * QT])
                nc.sync.dma_start(out=v1_sb[1], in_=v1_view[:, NKT // 2:NKT, :])
                nc.sync.dma_start(out=qt_sb[3], in_=qt_view[:, :, 3 * QT:4 * QT])
                return kt_sb, qt_sb, v1_sb

            def stage1(state, qi, warm=None):
                """S^T = Kt.T @ Qt for one 512-wide q tile; exp -> P^T bf16."""
                kt_sb, qt_sb, _ = state
                p_sb = p_pool.tile([128, NKT * QT], _BF16, tag="p")
                for g in range(NKG):
                    ps = ps_s.tile([128, KG * QT], _F32, tag="s")
                    for h in range(KG):
                        kj = g * KG + h
                        started = False
                        if g == 0 and h == 0 and warm is not None:
                            # HAM warm-up during the initial DMA wait:
                            # zero-matmuls accumulating 0 into this group.
                            for w in range(WARMUP_MMS):
                                nc.tensor.matmul(
                                    ps[:, h * QT:(h + 1) * QT],
                                    lhsT=warm[:, :128], rhs=warm,
                                    start=(w == 0), stop=False)
                            started = True
                        kh, ko = kj // 4, kj % 4
                        for c in range(NDC):
                            nc.tensor.matmul(
                                ps[:, h * QT:(h + 1) * QT],
                                lhsT=kt_sb[kh][:, c, ko * KT:(ko + 1) * KT],
                                rhs=qt_sb[qi][:, c, :],
                                start=(c == 0 and not started),
                                stop=(c == NDC - 1),
                            )
                    nc.scalar.activation(
                        out=p_sb[:, g * KG * QT:(g + 1) * KG * QT], in_=ps,
                        func=mybir.ActivationFunctionType.Exp,
                        scale=1.0 / 16.0)
                return p_sb

            def stage2(state, b, qi, p_sb):
                """O = P @ [V|1]; normalize by the ones column; DMA out."""
                _, _, v1_sb = state
                for s in range(NQS):
                    o_ps = ps_o.tile([128, VF], _F32, tag="o")
                    for kj in range(NKT):
                        nc.tensor.matmul(
                            o_ps,
                            lhsT=p_sb[:, kj * QT + s * QS:kj * QT + (s + 1) * QS],
                            rhs=v1_sb[kj // (NKT // 2)][:, kj % (NKT // 2), :],
                            start=(kj == 0), stop=(kj == NKT - 1),
                        )
                    recip = small_pool.tile([128, 1], _F32, tag="r")
                    nc.vector.reciprocal(out=recip, in_=o_ps[:, DV:DV + 1])
                    o_sb = o_pool.tile([128, DV], _F32, tag="o_sb")
                    nc.vector.tensor_scalar_mul(
                        out=o_sb, in0=o_ps[:, :DV], scalar1=recip)
                    q0 = qi * QT + s * QS
                    nc.sync.dma_start(out=out_d[b, q0:q0 + QS, :], in_=o_sb)

            warm = small_pool.tile([128, QT], _BF16, tag="warm")
            nc.vector.memset(warm, 0.0)

            states = [load_batch(b) for b in range(BPC)]
            work = [(b, qi) for b in range(BPC) for qi in range(NQT)]
            pending = None  # (state, b, qi, p_sb)
            for b, qi in work:
                p_sb = stage1(states[b], qi,
                              warm=warm if (b == 0 and qi == 0) else None)
                if pending is not None:
                    stage2(*pending)
                pending = (states[b], b, qi, p_sb)
            stage2(*pending)

    nc.compile()
    return nc


def _get_nc():
    if "nc" not in _NC_CACHE:
        _NC_CACHE["nc"] = _build_nc()
    return _NC_CACHE["nc"]


def _prepare(query, key, value, valid_length):
    query = np.asarray(query, dtype=np.float32)
    key = np.asarray(key, dtype=np.float32)
    value = np.asarray(value, dtype=np.float32)
    valid_length = np.asarray(valid_length)

    kz = key.copy()
    for b in range(B):
        kz[b, int(valid_length[b]):, :] = 0.0

    bf16 = ml_dtypes.bfloat16
    qt = np.ascontiguousarray(query.transpose(0, 2, 1)).astype(bf16)
    kt = np.ascontiguousarray(kz.transpose(0, 2, 1)).astype(bf16)
    v1 = np.concatenate(
        [value, np.ones((B, LK, 1), np.float32)], axis=-1).astype(bf16)
    return qt, kt, v1


def _run(inputs, trace=False):
    qt, kt, v1 = _prepare(**inputs)
    in_maps = [
        {"qt": qt[c * BPC:(c + 1) * BPC],
         "kt": kt[c * BPC:(c + 1) * BPC],
         "v1": v1[c * BPC:(c + 1) * BPC]}
        for c in range(N_CORES)
    ]
    nc = _get_nc()
    res = run_bass_kernel_spmd(nc, in_maps, core_ids=list(range(N_CORES)),
                               trace=trace)
    out = np.empty((B, LQ, DV), np.float32)
    for c in range(N_CORES):
        out[c * BPC:(c + 1) * BPC] = res.results[c]["out"]
    return out, res


def kernel(query, key, value, valid_length):
    out, _ = _run(dict(query=query, key=key, value=value,
                       valid_length=valid_length))
    return out


# revision 15
# speedup vs baseline: 1.0204x; 1.0204x over previous
"""Masked cross-attention kernel for Trainium2 (8 NeuronCores, SPMD).

Problem: B=16 batches of softmax(mask(Q@K^T/sqrt(D)))@V with
Lq=Lk=2048, D=DV=256.  The reference zeroes masked scores (NOT -inf)
before the softmax, so masked keys still contribute exp(0)=1 to the
denominator and weight 1/denom on V rows.

Strategy (all host prep is exact):
  * Zero K rows at k >= valid_length[b] on the host.  Then Q @ K^T is
    *exactly* 0.0 at masked positions — identical to the reference's
    jnp.where — and no mask tensor is needed on-device.
  * Pre-transpose Q and K to [D, L] layout on the host so both matmul
    operands stream naturally (contraction on the partition dim).
  * Append a ones-column to V.  P @ [V | 1] then yields the softmax
    denominator as output column 256 for free.
  * bf16 matmul inputs (fp32 PSUM accumulate), fp32 softmax math.

Per core: 2 batches.  Per batch, for each 512-wide q tile:
  stage 1: S^T[k,q] tiles in PSUM (KtT.T @ Qt), exp via ScalarE
           (scale=1/16 folded in) -> P^T bf16 in SBUF
  stage 2: O[q,v] = (P^T).T @ [V|1] accumulated over k chunks in PSUM;
           divide by column 256 (DVE reciprocal + per-partition mul).
Stage 1 of q-tile i+1 is emitted before stage 2 of q-tile i so the PE
never stalls on the ScalarE exp chain.
"""

import numpy as np
import ml_dtypes

import concourse.bass as bass
import concourse.mybir as mybir
import concourse.tile as tile
from concourse import bacc
from concourse.bass_utils import run_bass_kernel_spmd

B, LQ, LK, D, DV = 16, 2048, 2048, 256, 256
N_CORES = 8
BPC = B // N_CORES  # batches per core

QT = 512            # q-tile width (stage-1 moving free dim)
NQT = LQ // QT      # 4
KT = 128            # k-tile (partition dim of S^T)
NKT = LK // KT      # 16
KG = 2              # k-tiles per PSUM/exp group
NKG = NKT // KG     # 8
NDC = D // 128      # contraction chunks (2)
QS = 128            # q-subtile for stage 2
NQS = QT // QS      # 4
VF = DV + 1         # 257: V plus the ones column
WARMUP_MMS = 8      # HAM warm-up zero-matmuls before the first real MM

_BF16 = mybir.dt.bfloat16
_F32 = mybir.dt.float32

_NC_CACHE = {}


def _build_nc():
    nc = bacc.Bacc("TRN2", target_bir_lowering=False, debug=False,
                   num_devices=N_CORES)

    qt_d = nc.declare_dram_parameter("qt", [BPC, D, LQ], _BF16, isOutput=False)
    kt_d = nc.declare_dram_parameter("kt", [BPC, D, LK], _BF16, isOutput=False)
    v1_d = nc.declare_dram_parameter("v1", [BPC, LK, VF], _BF16, isOutput=False)
    out_d = nc.declare_dram_parameter("out", [BPC, LQ, DV], _F32, isOutput=True)

    with tile.TileContext(nc) as tc:
        with (
            tc.tile_pool(name="qk", bufs=2) as qk_pool,
            tc.tile_pool(name="v", bufs=2) as v_pool,
            tc.tile_pool(name="p", bufs=2) as p_pool,
            tc.tile_pool(name="osb", bufs=4) as o_pool,
            tc.tile_pool(name="small", bufs=8) as small_pool,
            tc.tile_pool(name="ps_s", bufs=2, space="PSUM") as ps_s,
            tc.tile_pool(name="ps_o", bufs=4, space="PSUM") as ps_o,
        ):
            def load_batch(b):
                # Split loads so the first q-tile's operands land ASAP:
                # kt per d-chunk, qt per q-tile; v1 (stage-2 only) after
                # the first q chunk.
                # kt split [c-chunk][k-half], v1 split in k-halves, qt per
                # q-tile: chunks land in the order the PE consumes them.
                QLK = LK // 4
                kt_view = kt_d[b].rearrange("(c p) k -> p c k", p=128)
                kt_sb = [qk_pool.tile([128, NDC, QLK], _BF16, tag=f"ktq{h}",
                                      name=f"ktq{h}_b{b}") for h in range(4)]
                qt_view = qt_d[b].rearrange("(c p) q -> p c q", p=128)
                qt_sb = [qk_pool.tile([128, NDC, QT], _BF16, tag=f"qt{qi}",
                                      name=f"qt{qi}_b{b}")
                         for qi in range(NQT)]
                v1_view = v1_d[b].rearrange("(t p) v -> p t v", p=128)
                v1_sb = [v_pool.tile([128, NKT // 2, VF], _BF16, tag=f"v1{h}",
                                     name=f"v1{h}_b{b}") for h in range(2)]

                # Two parallel DMA streams for batch 0 (latency-critical):
                # gpsimd carries ktq0/ktq1/qt1 concurrently with sync's
                # qt0/ktq2/ktq3/v1a/... so the PE never waits on one stream.
                # Batch 1 has no deadline - all sync.
                kq = [kt_view[:, :, h * QLK:(h + 1) * QLK] for h in range(4)]
                qv = [qt_view[:, :, qi * QT:(qi + 1) * QT] for qi in range(NQT)]
                vh = [v1_view[:, 0:NKT // 2, :], v1_view[:, NKT // 2:NKT, :]]
                if b == 0:
                    nc.gpsimd.dma_start(out=kt_sb[0], in_=kq[0])
                    nc.sync.dma_start(out=qt_sb[0], in_=qv[0])
                    nc.gpsimd.dma_start(out=kt_sb[1], in_=kq[1])
                    nc.sync.dma_start(out=kt_sb[2], in_=kq[2])
                    nc.gpsimd.dma_start(out=qt_sb[1], in_=qv[1])
                    nc.sync.dma_start(out=kt_sb[3], in_=kq[3])
                    nc.sync.dma_start(out=v1_sb[0], in_=vh[0])
                    nc.sync.dma_start(out=qt_sb[2], in_=qv[2])
                    nc.sync.dma_start(out=v1_sb[1], in_=vh[1])
                    nc.sync.dma_start(out=qt_sb[3], in_=qv[3])
                else:
                    for h in range(4):
                        nc.sync.dma_start(out=kt_sb[h], in_=kq[h])
                    nc.sync.dma_start(out=qt_sb[0], in_=qv[0])
                    nc.sync.dma_start(out=qt_sb[1], in_=qv[1])
                    nc.sync.dma_start(out=v1_sb[0], in_=vh[0])
                    nc.sync.dma_start(out=qt_sb[2], in_=qv[2])
                    nc.sync.dma_start(out=v1_sb[1], in_=vh[1])
                    nc.sync.dma_start(out=qt_sb[3], in_=qv[3])
                return kt_sb, qt_sb, v1_sb

            def stage1(state, qi, warm=None):
                """S^T = Kt.T @ Qt for one 512-wide q tile; exp -> P^T bf16."""
                kt_sb, qt_sb, _ = state
                p_sb = p_pool.tile([128, NKT * QT], _BF16, tag="p")
                for g in range(NKG):
                    ps = ps_s.tile([128, KG * QT], _F32, tag="s")
                    for h in range(KG):
                        kj = g * KG + h
                        started = False
                        if g == 0 and h == 0 and warm is not None:
                            # HAM warm-up during the initial DMA wait:
                            # zero-matmuls accumulating 0 into this group.
                            for w in range(WARMUP_MMS):
                                nc.tensor.matmul(
                                    ps[:, h * QT:(h + 1) * QT],
                                    lhsT=warm[:, :128], rhs=warm,
                                    start=(w == 0), stop=False)
                            started = True
                        kh, ko = kj // 4, kj % 4
                        for c in range(NDC):
                            nc.tensor.matmul(
                                ps[:, h * QT:(h + 1) * QT],
                                lhsT=kt_sb[kh][:, c, ko * KT:(ko + 1) * KT],
                                rhs=qt_sb[qi][:, c, :],
                                start=(c == 0 and not started),
                                stop=(c == NDC - 1),
                            )
                    nc.scalar.activation(
                        out=p_sb[:, g * KG * QT:(g + 1) * KG * QT], in_=ps,
                        func=mybir.ActivationFunctionType.Exp,
                        scale=1.0 / 16.0)
                return p_sb

            def stage2(state, b, qi, p_sb):
                """O = P @ [V|1]; normalize by the ones column; DMA out."""
                _, _, v1_sb = state
                for s in range(NQS):
                    o_ps = ps_o.tile([128, VF], _F32, tag="o")
                    for kj in range(NKT):
                        nc.tensor.matmul(
                            o_ps,
                            lhsT=p_sb[:, kj * QT + s * QS:kj * QT + (s + 1) * QS],
                            rhs=v1_sb[kj // (NKT // 2)][:, kj % (NKT // 2), :],
                            start=(kj == 0), stop=(kj == NKT - 1),
                        )
                    recip = small_pool.tile([128, 1], _F32, tag="r")
                    nc.vector.reciprocal(out=recip, in_=o_ps[:, DV:DV + 1])
                    o_sb = o_pool.tile([128, DV], _F32, tag="o_sb")
                    nc.vector.tensor_scalar_mul(
                        out=o_sb, in0=o_ps[:, :DV], scalar1=recip)
                    q0 = qi * QT + s * QS
                    nc.sync.dma_start(out=out_d[b, q0:q0 + QS, :], in_=o_sb)

            warm = small_pool.tile([128, QT], _BF16, tag="warm")
            nc.vector.memset(warm, 0.0)

            states = [load_batch(b) for b in range(BPC)]
            work = [(b, qi) for b in range(BPC) for qi in range(NQT)]
            pending = None  # (state, b, qi, p_sb)
            for b, qi in work:
                p_sb = stage1(states[b], qi,
                              warm=warm if (b == 0 and qi == 0) else None)
                if pending is not None:
                    stage2(*pending)
                pending = (states[b], b, qi, p_sb)
            stage2(*pending)

    nc.compile()
    return nc


def _get_nc():
    if "nc" not in _NC_CACHE:
        _NC_CACHE["nc"] = _build_nc()
    return _NC_CACHE["nc"]


def _prepare(query, key, value, valid_length):
    query = np.asarray(query, dtype=np.float32)
    key = np.asarray(key, dtype=np.float32)
    value = np.asarray(value, dtype=np.float32)
    valid_length = np.asarray(valid_length)

    kz = key.copy()
    for b in range(B):
        kz[b, int(valid_length[b]):, :] = 0.0

    bf16 = ml_dtypes.bfloat16
    qt = np.ascontiguousarray(query.transpose(0, 2, 1)).astype(bf16)
    kt = np.ascontiguousarray(kz.transpose(0, 2, 1)).astype(bf16)
    v1 = np.concatenate(
        [value, np.ones((B, LK, 1), np.float32)], axis=-1).astype(bf16)
    return qt, kt, v1


def _run(inputs, trace=False):
    qt, kt, v1 = _prepare(**inputs)
    in_maps = [
        {"qt": qt[c * BPC:(c + 1) * BPC],
         "kt": kt[c * BPC:(c + 1) * BPC],
         "v1": v1[c * BPC:(c + 1) * BPC]}
        for c in range(N_CORES)
    ]
    nc = _get_nc()
    res = run_bass_kernel_spmd(nc, in_maps, core_ids=list(range(N_CORES)),
                               trace=trace)
    out = np.empty((B, LQ, DV), np.float32)
    for c in range(N_CORES):
        out[c * BPC:(c + 1) * BPC] = res.results[c]["out"]
    return out, res


def kernel(query, key, value, valid_length):
    out, _ = _run(dict(query=query, key=key, value=value,
                       valid_length=valid_length))
    return out


# revision 16
# speedup vs baseline: 1.0272x; 1.0066x over previous
"""Masked cross-attention kernel for Trainium2 (8 NeuronCores, SPMD).

Problem: B=16 batches of softmax(mask(Q@K^T/sqrt(D)))@V with
Lq=Lk=2048, D=DV=256.  The reference zeroes masked scores (NOT -inf)
before the softmax, so masked keys still contribute exp(0)=1 to the
denominator and weight 1/denom on V rows.

Strategy (all host prep is exact):
  * Zero K rows at k >= valid_length[b] on the host.  Then Q @ K^T is
    *exactly* 0.0 at masked positions - identical to the reference's
    jnp.where - and no mask tensor is needed on-device.
  * Pre-transpose Q and K to [D, L] layout on the host so both matmul
    operands stream naturally (contraction on the partition dim).
  * Append a ones-column to V.  P @ [V | 1] then yields the softmax
    denominator as output column 256 for free.
  * bf16 matmul inputs (fp32 PSUM accumulate), fp32 softmax math.
  * All per-batch inputs are packed host-side into ONE blob tensor
    [128 partitions x cols] and loaded in 3 big segment DMAs (a single
    DMA fans out across all 16 SDMA engines; many small DMAs each pay
    a ~2us completion latency and fair-share the engines).

Per core: 2 batches.  Per batch, for each 512-wide q tile:
  stage 1: S^T[k,q] tiles in PSUM (Kt.T @ Qt), exp via ScalarE
           (scale=1/16 folded in) -> P^T bf16 in SBUF
  stage 2: O[q,v] = (P^T).T @ [V|1] accumulated over k chunks in PSUM;
           divide by column 256 (DVE reciprocal + per-partition mul).
Stage 1 of q-tile i+1 is emitted before stage 2 of q-tile i so the PE
never stalls on the ScalarE exp chain.
"""

import numpy as np
import ml_dtypes

import concourse.bass as bass
import concourse.mybir as mybir
import concourse.tile as tile
from concourse import bacc
from concourse.bass_utils import run_bass_kernel_spmd

B, LQ, LK, D, DV = 16, 2048, 2048, 256, 256
N_CORES = 8
BPC = B // N_CORES  # batches per core

QT = 512            # q-tile width (stage-1 moving free dim)
NQT = LQ // QT      # 4
KT = 128            # k-tile (partition dim of S^T)
NKT = LK // KT      # 16
KG = 2              # k-tiles per PSUM/exp group
NKG = NKT // KG     # 8
NDC = D // 128      # contraction chunks (2)
QS = 128            # q-subtile for stage 2
NQS = QT // QS      # 4
VF = DV + 1         # 257: V plus the ones column
WARMUP_MMS = 8      # HAM warm-up zero-matmuls before the first real MM

# Blob column layout (per partition, bf16):
#   seg A: kt (c,k): NDC*LK = 4096   | qt0 (c,q): NDC*QT = 1024  -> 5120
#   seg B: qt1: 1024                 | v1a (t,v): 8*VF = 2056    -> 3080
#   seg C: qt2: 1024 | v1b: 2056 | qt3: 1024                     -> 4104
SEG_A = NDC * LK + NDC * QT          # 5120
SEG_B = NDC * QT + (NKT // 2) * VF   # 3080
SEG_C = 2 * NDC * QT + (NKT // 2) * VF  # 4104
BLOB = SEG_A + SEG_B + SEG_C         # 12304

_BF16 = mybir.dt.bfloat16
_F32 = mybir.dt.float32

_NC_CACHE = {}


def _build_nc():
    nc = bacc.Bacc("TRN2", target_bir_lowering=False, debug=False,
                   num_devices=N_CORES)

    blob_d = nc.declare_dram_parameter("blob", [BPC, 128, BLOB], _BF16,
                                       isOutput=False)
    out_d = nc.declare_dram_parameter("out", [BPC, LQ, DV], _F32,
                                      isOutput=True)

    with tile.TileContext(nc) as tc:
        with (
            tc.tile_pool(name="seg", bufs=2) as seg_pool,
            tc.tile_pool(name="p", bufs=2) as p_pool,
            tc.tile_pool(name="osb", bufs=2) as o_pool,
            tc.tile_pool(name="small", bufs=8) as small_pool,
            tc.tile_pool(name="ps_s", bufs=2, space="PSUM") as ps_s,
            tc.tile_pool(name="ps_o", bufs=4, space="PSUM") as ps_o,
        ):
            def load_batch(b):
                segs = []
                for si, (lo, n) in enumerate(
                        [(0, SEG_A), (SEG_A, SEG_B), (SEG_A + SEG_B, SEG_C)]):
                    t = seg_pool.tile([128, n], _BF16, tag=f"seg{si}",
                                      name=f"seg{si}_b{b}")
                    nc.sync.dma_start(out=t, in_=blob_d[b, :, lo:lo + n])
                    segs.append(t)
                return segs

            def kt_slice(segs, c, kj):
                return segs[0][:, c * LK + kj * KT:c * LK + (kj + 1) * KT]

            def qt_slice(segs, qi, c):
                offs = [(0, NDC * LK), (1, 0), (2, 0), (2, NDC * QT + (NKT // 2) * VF)]
                si, o = offs[qi]
                return segs[si][:, o + c * QT:o + (c + 1) * QT]

            def v1_slice(segs, kj):
                if kj < NKT // 2:
                    return segs[1][:, NDC * QT + kj * VF:NDC * QT + (kj + 1) * VF]
                kj -= NKT // 2
                return segs[2][:, NDC * QT + kj * VF:NDC * QT + (kj + 1) * VF]

            def stage1(segs, qi, warm=None):
                """S^T = Kt.T @ Qt for one 512-wide q tile; exp -> P^T bf16."""
                p_sb = p_pool.tile([128, NKT * QT], _BF16, tag="p")
                for g in range(NKG):
                    ps = ps_s.tile([128, KG * QT], _F32, tag="s")
                    for h in range(KG):
                        kj = g * KG + h
                        started = False
                        if g == 0 and h == 0 and warm is not None:
                            # HAM warm-up during the initial DMA wait:
                            # zero-matmuls accumulating 0 into this group.
                            for w in range(WARMUP_MMS):
                                nc.tensor.matmul(
                                    ps[:, h * QT:(h + 1) * QT],
                                    lhsT=warm[:, :128], rhs=warm,
                                    start=(w == 0), stop=False)
                            started = True
                        for c in range(NDC):
                            nc.tensor.matmul(
                                ps[:, h * QT:(h + 1) * QT],
                                lhsT=kt_slice(segs, c, kj),
                                rhs=qt_slice(segs, qi, c),
                                start=(c == 0 and not started),
                                stop=(c == NDC - 1),
                            )
                    nc.scalar.activation(
                        out=p_sb[:, g * KG * QT:(g + 1) * KG * QT], in_=ps,
                        func=mybir.ActivationFunctionType.Exp,
                        scale=1.0 / 16.0)
                return p_sb

            def stage2(segs, b, qi, p_sb):
                """O = P @ [V|1]; normalize by the ones column; DMA out."""
                o_sb = o_pool.tile([128, NQS, DV], _F32, tag="o_sb")
                for s in range(NQS):
                    o_ps = ps_o.tile([128, VF], _F32, tag="o")
                    for kj in range(NKT):
                        nc.tensor.matmul(
                            o_ps,
                            lhsT=p_sb[:, kj * QT + s * QS:kj * QT + (s + 1) * QS],
                            rhs=v1_slice(segs, kj),
                            start=(kj == 0), stop=(kj == NKT - 1),
                        )
                    recip = small_pool.tile([128, 1], _F32, tag="r")
                    nc.vector.reciprocal(out=recip, in_=o_ps[:, DV:DV + 1])
                    nc.vector.tensor_scalar_mul(
                        out=o_sb[:, s, :], in0=o_ps[:, :DV], scalar1=recip)
                out_view = out_d[b, qi * QT:(qi + 1) * QT, :].rearrange(
                    "(s p) v -> p s v", p=128)
                nc.sync.dma_start(out=out_view, in_=o_sb)

            warm = small_pool.tile([128, QT], _BF16, tag="warm")
            nc.vector.memset(warm, 0.0)

            states = [load_batch(b) for b in range(BPC)]
            work = [(b, qi) for b in range(BPC) for qi in range(NQT)]
            pending = None  # (segs, b, qi, p_sb)
            for b, qi in work:
                p_sb = stage1(states[b], qi,
                              warm=warm if (b == 0 and qi == 0) else None)
                if pending is not None:
                    stage2(*pending)
                pending = (states[b], b, qi, p_sb)
            stage2(*pending)

    nc.compile()
    return nc


def _get_nc():
    if "nc" not in _NC_CACHE:
        _NC_CACHE["nc"] = _build_nc()
    return _NC_CACHE["nc"]


def _prepare(query, key, value, valid_length):
    query = np.asarray(query, dtype=np.float32)
    key = np.asarray(key, dtype=np.float32)
    value = np.asarray(value, dtype=np.float32)
    valid_length = np.asarray(valid_length)

    kz = key.copy()
    for b in range(B):
        kz[b, int(valid_length[b]):, :] = 0.0

    bf16 = ml_dtypes.bfloat16
    # kt[b, p, c*LK + k] = Kz[b, k, c*128+p]
    kt = kz.transpose(0, 2, 1).reshape(B, NDC, 128, LK) \
        .transpose(0, 2, 1, 3).reshape(B, 128, NDC * LK)
    # qt_q[qi][b, p, c*QT + q'] = Q[b, qi*QT+q', c*128+p]
    qarr = query.transpose(0, 2, 1).reshape(B, NDC, 128, LQ) \
        .transpose(0, 2, 1, 3)  # [B, 128, NDC, LQ]
    qts = [qarr[:, :, :, qi * QT:(qi + 1) * QT].reshape(B, 128, NDC * QT)
           for qi in range(NQT)]
    # v1h[b, p, t*VF + v] = [V|1][b, (8h+t)*128+p, v]
    v1 = np.concatenate(
        [value, np.ones((B, LK, 1), np.float32)], axis=-1)  # [B, LK, VF]
    v1arr = v1.reshape(B, NKT, 128, VF).transpose(0, 2, 1, 3)  # [B,128,NKT,VF]
    v1a = v1arr[:, :, 0:NKT // 2, :].reshape(B, 128, (NKT // 2) * VF)
    v1b = v1arr[:, :, NKT // 2:NKT, :].reshape(B, 128, (NKT // 2) * VF)

    blob = np.concatenate(
        [kt, qts[0], qts[1], v1a, qts[2], v1b, qts[3]], axis=2)
    assert blob.shape == (B, 128, BLOB)
    return np.ascontiguousarray(blob).astype(bf16)


def _run(inputs, trace=False):
    blob = _prepare(**inputs)
    in_maps = [{"blob": blob[c * BPC:(c + 1) * BPC]} for c in range(N_CORES)]
    nc = _get_nc()
    res = run_bass_kernel_spmd(nc, in_maps, core_ids=list(range(N_CORES)),
                               trace=trace)
    out = np.empty((B, LQ, DV), np.float32)
    for c in range(N_CORES):
        out[c * BPC:(c + 1) * BPC] = res.results[c]["out"]
    return out, res


def kernel(query, key, value, valid_length):
    out, _ = _run(dict(query=query, key=key, value=value,
                       valid_length=valid_length))
    return out


# revision 20
# speedup vs baseline: 1.0401x; 1.0126x over previous
"""Masked cross-attention kernel for Trainium2 (8 NeuronCores, SPMD).

Problem: B=16 batches of softmax(mask(Q@K^T/sqrt(D)))@V with
Lq=Lk=2048, D=DV=256.  The reference zeroes masked scores (NOT -inf)
before the softmax, so masked keys still contribute exp(0)=1 to the
denominator and weight 1/denom on V rows.

Strategy (all host prep is exact):
  * Zero K rows at k >= valid_length[b] on the host.  Then Q @ K^T is
    *exactly* 0.0 at masked positions - identical to the reference's
    jnp.where - and no mask tensor is needed on-device.
  * Pre-transpose Q and K to [D, L] layout on the host so both matmul
    operands stream naturally (contraction on the partition dim).
  * Append a ones-column to V.  P @ [V | 1] then yields the softmax
    denominator as output column 256 for free.
  * bf16 matmul inputs (fp32 PSUM accumulate), fp32 softmax math.
  * All per-batch inputs are packed host-side into ONE blob tensor
    [128 partitions x cols] and loaded in 3 big segment DMAs (a single
    DMA fans out across all 16 SDMA engines; many small DMAs each pay
    a ~2us completion latency and fair-share the engines).

Per core: 2 batches.  Per batch, for each 512-wide q tile:
  stage 1: S^T[k,q] tiles in PSUM (Kt.T @ Qt), exp via ScalarE
           (scale=1/16 folded in) -> P^T bf16 in SBUF
  stage 2: O[q,v] = (P^T).T @ [V|1] accumulated over k chunks in PSUM;
           divide by column 256 (DVE reciprocal + per-partition mul).
Stage 1 of q-tile i+1 is emitted before stage 2 of q-tile i so the PE
never stalls on the ScalarE exp chain.
"""

import numpy as np
import ml_dtypes

import concourse.bass as bass
import concourse.mybir as mybir
import concourse.tile as tile
from concourse import bacc
from concourse.bass_utils import run_bass_kernel_spmd

B, LQ, LK, D, DV = 16, 2048, 2048, 256, 256
N_CORES = 8
BPC = B // N_CORES  # batches per core

QT = 512            # q-tile width (stage-1 moving free dim)
NQT = LQ // QT      # 4
KT = 128            # k-tile (partition dim of S^T)
NKT = LK // KT      # 16
KG = 2              # k-tiles per PSUM/exp group
NKG = NKT // KG     # 8
NDC = D // 128      # contraction chunks (2)
QS = 128            # q-subtile for stage 2
NQS = QT // QS      # 4
VF = DV + 1         # 257: V plus the ones column
WARMUP_MMS = 8      # HAM warm-up zero-matmuls before the first real MM

# Blob column layout (per partition, bf16).  Segments sized so the
# latency-critical first working set (kt + qt0) splits evenly across
# the two independent HWDGE rings (sync + scalar), which run FIFO-serial
# per ring at ~170GB/s each:
#   seg A1: kt_c0 2048 | qt0_c0 512   -> 2560   (sync)
#   seg A2: kt_c1 2048 | qt0_c1 512   -> 2560   (scalar)
#   seg B:  qt1 1024   | v1a 8*VF=2056 -> 3080  (sync)
#   seg C:  qt2 1024   | v1b 2056      -> 3080  (scalar)
#   seg D:  qt3 1024                   -> 1024  (sync)
SEG_A = LK + QT                      # 2560 (x2)
SEG_B = NDC * QT + (NKT // 2) * VF   # 3080
SEG_C = SEG_B                        # 3080
SEG_D = NDC * QT                     # 1024
BLOB = 2 * SEG_A + SEG_B + SEG_C + SEG_D  # 12304

_BF16 = mybir.dt.bfloat16
_F32 = mybir.dt.float32

_NC_CACHE = {}


def _build_nc():
    nc = bacc.Bacc("TRN2", target_bir_lowering=False, debug=False,
                   num_devices=N_CORES)

    blob_d = nc.declare_dram_parameter("blob", [BPC, 128, BLOB], _BF16,
                                       isOutput=False)
    out_d = nc.declare_dram_parameter("out", [BPC, LQ, DV], _F32,
                                      isOutput=True)

    with tile.TileContext(nc) as tc:
        with (
            tc.tile_pool(name="seg", bufs=2) as seg_pool,
            tc.tile_pool(name="p", bufs=2) as p_pool,
            tc.tile_pool(name="osb", bufs=2) as o_pool,
            tc.tile_pool(name="small", bufs=8) as small_pool,
            tc.tile_pool(name="ps_s", bufs=2, space="PSUM") as ps_s,
            tc.tile_pool(name="ps_o", bufs=4, space="PSUM") as ps_o,
        ):
            SEG_TBL = [  # (offset, size)
                (0, SEG_A), (SEG_A, SEG_A), (2 * SEG_A, SEG_B),
                (2 * SEG_A + SEG_B, SEG_C), (2 * SEG_A + SEG_B + SEG_C, SEG_D),
            ]

            def load_batch(b):
                segs = []
                # batch 0 is latency-critical: A1/B/D on the sync ring,
                # A2/C on the scalar ring, concurrently.  batch 1 loads
                # all on sync (no deadline; keeps ACT free for exps).
                engs = ([nc.sync, nc.scalar, nc.sync, nc.scalar, nc.sync]
                        if b == 0 else [nc.sync] * 5)
                for si, (lo, n) in enumerate(SEG_TBL):
                    t = seg_pool.tile([128, n], _BF16, tag=f"seg{si}",
                                      name=f"seg{si}_b{b}")
                    engs[si].dma_start(out=t, in_=blob_d[b, :, lo:lo + n])
                    segs.append(t)
                return segs

            def kt_slice(segs, c, kj):
                return segs[c][:, kj * KT:(kj + 1) * KT]

            def qt_slice(segs, qi, c):
                if qi == 0:
                    return segs[c][:, LK:LK + QT]
                offs = [(2, 0), (3, 0), (4, 0)]
                si, o = offs[qi - 1]
                return segs[si][:, o + c * QT:o + (c + 1) * QT]

            def v1_slice(segs, kj):
                if kj < NKT // 2:
                    return segs[2][:, NDC * QT + kj * VF:NDC * QT + (kj + 1) * VF]
                kj -= NKT // 2
                return segs[3][:, NDC * QT + kj * VF:NDC * QT + (kj + 1) * VF]

            def stage1(segs, qi, warm=None):
                """S^T = Kt.T @ Qt for one 512-wide q tile; exp -> P^T bf16."""
                p_sb = p_pool.tile([128, NKT * QT], _BF16, tag="p")
                for g in range(NKG):
                    ps = ps_s.tile([128, KG * QT], _F32, tag="s")
                    for h in range(KG):
                        kj = g * KG + h
                        started = False
                        if g == 0 and h == 0 and warm is not None:
                            # HAM warm-up during the initial DMA wait:
                            # zero-matmuls accumulating 0 into this group.
                            for w in range(WARMUP_MMS):
                                nc.tensor.matmul(
                                    ps[:, h * QT:(h + 1) * QT],
                                    lhsT=warm[:, :128], rhs=warm,
                                    start=(w == 0), stop=False)
                            started = True
                        for c in range(NDC):
                            nc.tensor.matmul(
                                ps[:, h * QT:(h + 1) * QT],
                                lhsT=kt_slice(segs, c, kj),
                                rhs=qt_slice(segs, qi, c),
                                start=(c == 0 and not started),
                                stop=(c == NDC - 1),
                            )
                    nc.scalar.activation(
                        out=p_sb[:, g * KG * QT:(g + 1) * KG * QT], in_=ps,
                        func=mybir.ActivationFunctionType.Exp,
                        scale=1.0 / 16.0)
                return p_sb

            def stage2(segs, b, qi, p_sb):
                """O = P @ [V|1]; normalize by the ones column; DMA out."""
                for s in range(NQS):
                    o_ps = ps_o.tile([128, VF], _F32, tag="o")
                    for kj in range(NKT):
                        nc.tensor.matmul(
                            o_ps,
                            lhsT=p_sb[:, kj * QT + s * QS:kj * QT + (s + 1) * QS],
                            rhs=v1_slice(segs, kj),
                            start=(kj == 0), stop=(kj == NKT - 1),
                        )
                    recip = small_pool.tile([128, 1], _F32, tag="r")
                    nc.vector.reciprocal(out=recip, in_=o_ps[:, DV:DV + 1])
                    o_sb = o_pool.tile([128, DV], _F32, tag="o_sb")
                    nc.vector.tensor_scalar_mul(
                        out=o_sb, in0=o_ps[:, :DV], scalar1=recip)
                    q0 = qi * QT + s * QS
                    nc.sync.dma_start(out=out_d[b, q0:q0 + QS, :], in_=o_sb)

            warm = small_pool.tile([128, QT], _BF16, tag="warm")
            nc.vector.memset(warm, 0.0)

            states = [load_batch(b) for b in range(BPC)]
            work = [(b, qi) for b in range(BPC) for qi in range(NQT)]
            pending = None  # (segs, b, qi, p_sb)
            for b, qi in work:
                p_sb = stage1(states[b], qi,
                              warm=warm if (b == 0 and qi == 0) else None)
                if pending is not None:
                    stage2(*pending)
                pending = (states[b], b, qi, p_sb)
            stage2(*pending)

    nc.compile()
    return nc


def _get_nc():
    if "nc" not in _NC_CACHE:
        _NC_CACHE["nc"] = _build_nc()
    return _NC_CACHE["nc"]


def _prepare(query, key, value, valid_length):
    query = np.asarray(query, dtype=np.float32)
    key = np.asarray(key, dtype=np.float32)
    value = np.asarray(value, dtype=np.float32)
    valid_length = np.asarray(valid_length)

    kz = key.copy()
    for b in range(B):
        kz[b, int(valid_length[b]):, :] = 0.0

    bf16 = ml_dtypes.bfloat16
    # ktc[c][b, p, k] = Kz[b, k, c*128+p]
    karr = kz.transpose(0, 2, 1).reshape(B, NDC, 128, LK)  # [B, c, p, k]
    # qc[b, p, c, q] = Q[b, q, c*128+p]
    qarr = query.transpose(0, 2, 1).reshape(B, NDC, 128, LQ) \
        .transpose(0, 2, 1, 3)  # [B, 128, NDC, LQ]
    qts = [qarr[:, :, :, qi * QT:(qi + 1) * QT].reshape(B, 128, NDC * QT)
           for qi in range(NQT)]
    # qt0 split per c-chunk for the A segments
    qt0c = [qarr[:, :, c, 0:QT] for c in range(NDC)]  # [B, 128, QT] each
    # v1h[b, p, t*VF + v] = [V|1][b, (8h+t)*128+p, v]
    v1 = np.concatenate(
        [value, np.ones((B, LK, 1), np.float32)], axis=-1)  # [B, LK, VF]
    v1arr = v1.reshape(B, NKT, 128, VF).transpose(0, 2, 1, 3)  # [B,128,NKT,VF]
    v1a = v1arr[:, :, 0:NKT // 2, :].reshape(B, 128, (NKT // 2) * VF)
    v1b = v1arr[:, :, NKT // 2:NKT, :].reshape(B, 128, (NKT // 2) * VF)

    blob = np.concatenate(
        [karr[:, 0], qt0c[0],          # seg A1
         karr[:, 1], qt0c[1],          # seg A2
         qts[1], v1a,                  # seg B
         qts[2], v1b,                  # seg C
         qts[3]],                      # seg D
        axis=2)
    assert blob.shape == (B, 128, BLOB)
    return np.ascontiguousarray(blob).astype(bf16)


def _run(inputs, trace=False):
    blob = _prepare(**inputs)
    in_maps = [{"blob": blob[c * BPC:(c + 1) * BPC]} for c in range(N_CORES)]
    nc = _get_nc()
    res = run_bass_kernel_spmd(nc, in_maps, core_ids=list(range(N_CORES)),
                               trace=trace)
    out = np.empty((B, LQ, DV), np.float32)
    for c in range(N_CORES):
        out[c * BPC:(c + 1) * BPC] = res.results[c]["out"]
    return out, res


def kernel(query, key, value, valid_length):
    out, _ = _run(dict(query=query, key=key, value=value,
                       valid_length=valid_length))
    return out
